# revision 22
# baseline (speedup 1.0000x reference)
"""Trainium2 Bass kernel for nn_CapsuleNetwork (MIND-style capsule routing).

Math (per batch row b):
  hat[s, kd]   = sum_d his[s, d] * w[s, kd, d]          (per-position bilinear)
  3 rounds of dynamic routing over K=4 interest capsules:
    sw = softmax_k(cw) * valid;  cap_k = sum_s sw[k,s] * hat[s,(k,:)]
    cap = squash(cap);           cw  += hat . cap       (first 2 rounds)
  outputs: interest_capsule [B,K,D], readout = capsule at argmax_k <cap_k, eb>

Distribution: pure data parallel over the batch across 8 NeuronCores
(128 rows/core, w replicated).

Precision strategy: the readout gathers the argmax capsule, so our capsules
must track an fp32 reference closely enough that no row's argmax flips
(~1e-5).  Every routing quantity is therefore computed in fp32:
  - masked his and w are split into bf16 hi+lo pairs (exact to ~2^-17);
    a DRAM round-trip through the 2-byte xbar DMA transpose produces the
    d-on-partition layouts [(s2 d), c, ...] the PE needs; the four
    transposed operand tensors stay resident in SBUF (~150KB/partition).
  - hat is never stored: each routing pass re-computes exact fp32 hat
    chunks on the (otherwise idle) PE as 3 accumulating bf16 matmuls
    (hi*hi + hi*lo + lo*hi) into PSUM, which the DVE consumes in fp32.
  - iteration-0's capsule needs no weights (sw0 uniform, mask folded into
    his) and comes from one K=128 accumulating matmul chain.
"""

import os

import numpy as np

import concourse.bass as bass
import concourse.mybir as mybir
import concourse.tile as tile
from concourse.vector_clock import ScopedClock

B, S, K, D = 1024, 200, 4, 64
NCORES = 8
BL = B // NCORES          # 128 batch rows per core
KD = K * D                # 256
C = S // 2                # 100 s-pair chunks
SG = 4                    # s-positions per PSUM-resident recompute group
NSG = S // SG             # 25 groups
F32 = mybir.dt.float32
BF16 = mybir.dt.bfloat16
I32 = mybir.dt.int32
MUL = mybir.AluOpType.mult
ADD = mybir.AluOpType.add
SUB = mybir.AluOpType.subtract
MAX = mybir.AluOpType.max
EQ = mybir.AluOpType.is_equal
AF = mybir.ActivationFunctionType
AX = mybir.AxisListType


class _TC(tile.TileContext):
    """This walrus build rejects >2 sync waits on one CTRL instruction, so
    spread the tail-drain's waits over one drain each."""

    def _drain_and_barrier(self, tick_clock, wait_clock):
        nc = self.nc
        drain_inst = nc.sync.drain()
        wait_clock.add_sem_waits(
            drain_inst.ins, ScopedClock({None: tick_clock.global_clock})
        )
        si = drain_inst.ins.sync_info
        if si is not None and si.on_wait and len(si.on_wait) > 1:
            waits = list(si.on_wait)
            si.on_wait = waits[:1]
            for w in waits[1:]:
                d2 = nc.sync.drain()
                s2 = d2.ins.sync_info
                if s2 is None:
                    d2.ins.sync_info = mybir.SyncInfo(on_wait=[w], on_update=[])
                else:
                    s2.on_wait = [w]
        nc.all_engine_barrier()
        assert self.sems is not None
        popped = nc._tile_sem_poison_stack.pop()
        assert popped is self._sem_poison
        # chunk the sem clears: this walrus build rejects the long-range
        # RANGE_CLEAR encoding ("ISA wrong length")
        allocated = sorted(
            s.num if hasattr(s, "num") else s
            for s in self.sems.allocated().values()
        )
        CH = 8
        for i in range(0, len(allocated), CH):
            nc.clear_and_free_semaphores(allocated[i : i + CH])
        nc.all_engine_barrier()

    @staticmethod
    def _split_excess_waits_static(nc, max_waits=1):
        """Walrus rejects instructions with too many sync waits; hoist the
        excess onto per-engine drain instructions inserted just before."""
        nid = [0]

        def visit(bb):
            il = bb.instructions
            i = 0
            while i < len(il):
                inst = il[i]
                if hasattr(inst, "blocks"):
                    for b2 in inst.blocks:
                        visit(b2)
                si = getattr(inst, "sync_info", None)
                if si is not None and si.on_wait and len(si.on_wait) > max_waits:
                    waits = list(si.on_wait)
                    si.on_wait = waits[:max_waits]
                    extra = waits[max_waits:]
                    for j in range(0, len(extra), max_waits):
                        nd = mybir.InstDrain(
                            name=f"splitw_{nid[0]}", ins=[], outs=[]
                        )
                        nid[0] += 1
                        nd.engine = inst.engine
                        nd.is_reset_sema = False
                        nd.debug = inst.debug
                        nd.sync_info = mybir.SyncInfo(
                            on_wait=extra[j : j + max_waits], on_update=[]
                        )
                        il.insert(i, nd)
                        i += 1
                i += 1

        for f in nc.m.functions:
            for bb in f.blocks:
                visit(bb)


def _squash_factor(nc, pool, cap, name):
    """f[b,k] = n/(1+n)/sqrt(n+1e-9) for n = |cap_k|^2; returns f32 [128,K]."""
    nvec = pool.tile([BL, K], F32, tag=f"n_{name}")
    junk = pool.tile([BL, D], F32, tag="sq_junk")
    for k in range(K):
        nc.scalar.activation(
            junk[:], cap[:, k, :], AF.Square, accum_out=nvec[:, k : k + 1]
        )
    # rsqrt(n + eps) = exp(-0.5 * ln(n + eps)) (ACT Rsqrt is banned/inaccurate)
    neps = pool.tile([BL, K], F32, tag="sq_ne")
    nc.vector.tensor_scalar_add(neps[:], nvec[:], 1e-9)
    lnv = pool.tile([BL, K], F32, tag="sq_ln")
    nc.scalar.activation(lnv[:], neps[:], AF.Ln)
    rsq = pool.tile([BL, K], F32, tag="sq_rs")
    nc.scalar.activation(rsq[:], lnv[:], AF.Exp, scale=-0.5)
    dn = pool.tile([BL, K], F32, tag="sq_dn")
    nc.vector.tensor_scalar_add(dn[:], nvec[:], 1.0)
    dnl = pool.tile([BL, K], F32, tag="sq_dnl")
    nc.scalar.activation(dnl[:], dn[:], AF.Ln)
    rdn = pool.tile([BL, K], F32, tag="sq_rd")
    nc.scalar.activation(rdn[:], dnl[:], AF.Exp, scale=-1.0)
    f = pool.tile([BL, K], F32, tag=f"f_{name}")
    nc.vector.tensor_tensor(f[:], nvec[:], rsq[:], MUL)
    nc.vector.tensor_tensor(f[:], f[:], rdn[:], MUL)
    return f


def _capq_from(nc, pool, cap, f, name):
    """capq[b,k,d] = cap * f (f32) - squashed capsule for routing."""
    capq = pool.tile([BL, K, D], F32, tag=f"capq_{name}")
    f_b = f[:].unsqueeze(2).broadcast_to([BL, K, D])
    nc.vector.tensor_tensor(capq[:], cap[:], f_b, MUL)
    return capq


def _softmax_sw(nc, pool, cw, mkf, name):
    """sw[b,k,s] = softmax_k(cw) * mask (f32), stored (k,s)."""
    mx = pool.tile([BL, S], F32, tag="sm_mx")
    nc.vector.tensor_tensor(mx[:], cw[:, 0, :], cw[:, 1, :], MAX)
    nc.vector.tensor_tensor(mx[:], mx[:], cw[:, 2, :], MAX)
    nc.vector.tensor_tensor(mx[:], mx[:], cw[:, 3, :], MAX)
    e = pool.tile([BL, K, S], F32, tag="sm_e")
    mx_b = mx[:].unsqueeze(1).broadcast_to([BL, K, S])
    nc.vector.tensor_tensor(e[:], cw[:], mx_b, SUB)
    nc.scalar.activation(e[:], e[:], AF.Exp)
    z = pool.tile([BL, S], F32, tag="sm_z")
    nc.vector.tensor_tensor(z[:], e[:, 0, :], e[:, 1, :], ADD)
    nc.vector.tensor_tensor(z[:], z[:], e[:, 2, :], ADD)
    nc.vector.tensor_tensor(z[:], z[:], e[:, 3, :], ADD)
    zl = pool.tile([BL, S], F32, tag="sm_zl")
    nc.scalar.activation(zl[:], z[:], AF.Ln)
    rz = pool.tile([BL, S], F32, tag="sm_rz")
    nc.scalar.activation(rz[:], zl[:], AF.Exp, scale=-1.0)
    nc.vector.tensor_tensor(rz[:], rz[:], mkf[:], MUL)  # fold mask in
    sw = pool.tile([BL, K, S], F32, tag=f"sw_{name}")
    rz_b = rz[:].unsqueeze(1).broadcast_to([BL, K, S])
    nc.vector.tensor_tensor(sw[:], e[:], rz_b, MUL)
    return sw


def build_bass():
    nc = bass.Bass()
    his = nc.declare_dram_parameter("his", [BL, S, D], F32, isOutput=False)
    msk = nc.declare_dram_parameter("msk", [BL, S], I32, isOutput=False)
    eb = nc.declare_dram_parameter("eb", [BL, D], F32, isOutput=False)
    w = nc.declare_dram_parameter("w", [S, KD, D], F32, isOutput=False)
    cap_out = nc.declare_dram_parameter("cap_out", [BL, K, D], F32, isOutput=True)
    ro_out = nc.declare_dram_parameter("ro_out", [BL, D], F32, isOutput=True)

    with _TC(nc) as tc:
        with (
            tc.tile_pool(name="persist", bufs=1) as P,
            tc.tile_pool(name="dram", bufs=1, space="DRAM") as DP,
        ):
            zc = P.tile([BL, 1], F32)
            nc.vector.memset(zc[:], 0.0)
            nc.const_aps.aps[(F32, 0.0)] = zc[:]
            mkf = P.tile([BL, S], F32)
            ebt = P.tile([BL, D], F32)
            cap0s = P.tile([BL, K, D], F32)

            # DRAM scratch: bf16 hi/lo staged for xbar transposes
            mh_scr = DP.tile([2, C, BL, 2, D], BF16)     # [hl, c, b, s2, d]
            w_scr = DP.tile([2, C, 2, 128, 2, D], BF16)  # [hl, c, h, p, s2, d]

            # ---------------- phase 0: stage hi/lo bf16 splits through DRAM
            with tc.tile_pool(name="ph0", bufs=2) as P0:
                mki = P0.tile([BL, S], I32, tag="mki", bufs=1)
                nc.sync.dma_start(mki[:], msk[:])
                nc.vector.tensor_copy(mkf[:], mki[:])  # int32 -> f32 (0/1)
                nc.sync.dma_start(ebt[:], eb[:])

                Q = 4
                SQ = S // Q  # 50
                for q in range(Q):
                    s0 = q * SQ
                    hb = P0.tile([BL, SQ, D], F32, tag="hisb")
                    nc.sync.dma_start(hb[:], his[:, s0 : s0 + SQ, :])
                    mask_b = (
                        mkf[:, s0 : s0 + SQ].unsqueeze(2).broadcast_to([BL, SQ, D])
                    )
                    mhi = P0.tile([BL, SQ, D], BF16, tag="mhi")
                    nc.scalar.copy(mhi[:], hb[:])
                    mlo = P0.tile([BL, SQ, D], BF16, tag="mlo")
                    nc.vector.tensor_tensor(mlo[:], hb[:], mhi[:], SUB)
                    # mask is 0/1 so masking commutes with the bf16 split
                    nc.vector.tensor_tensor(mhi[:], mhi[:], mask_b, MUL)
                    nc.vector.tensor_tensor(mlo[:], mlo[:], mask_b, MUL)
                    for hl, tt in ((0, mhi), (1, mlo)):
                        dst = mh_scr[
                            hl, q * (SQ // 2) : (q + 1) * (SQ // 2)
                        ].rearrange("c b t d -> b c t d")
                        nc.sync.dma_start(
                            dst, tt[:].rearrange("b (c t) d -> b c t d", t=2)
                        )

                WB = 10
                SB = S // WB  # 20 positions per block
                for wbk in range(WB):
                    s0 = wbk * SB
                    c0 = s0 // 2
                    for h in range(2):
                        wt = P0.tile([128, SB // 2, 2, D], F32, tag="wtmp")
                        src = w[s0 : s0 + SB, 128 * h : 128 * h + 128, :].rearrange(
                            "(c t) p d -> p c t d", t=2
                        )
                        nc.sync.dma_start(wt[:], src)
                        whi = P0.tile([128, SB // 2, 2, D], BF16, tag="whi")
                        nc.scalar.copy(whi[:], wt[:])
                        wlo = P0.tile([128, SB // 2, 2, D], BF16, tag="wlo")
                        nc.vector.tensor_tensor(wlo[:], wt[:], whi[:], SUB)
                        for hl, tt in ((0, whi), (1, wlo)):
                            dst = w_scr[hl, c0 : c0 + SB // 2, h].rearrange(
                                "c p t d -> p c t d"
                            )
                            nc.sync.dma_start(dst, tt[:])

            # ---------------- resident transposed operands + routing
            with (
                tc.tile_pool(name="ops", bufs=1) as OP,
                tc.tile_pool(name="rt", bufs=1) as R,
                tc.tile_pool(name="rt2", bufs=1) as R2,
                tc.tile_pool(name="psumg", bufs=2, space="PSUM") as PSG,
            ):
                mhT = [
                    OP.tile([128, C, BL], BF16, tag=f"mhT{hl}", name=f"mhT{hl}")
                    for hl in range(2)
                ]
                wT = [
                    OP.tile([128, C, KD], BF16, tag=f"wT{hl}", name=f"wT{hl}")
                    for hl in range(2)
                ]
                for hl in range(2):
                    for c in range(C):
                        nc.sync.dma_start_transpose(
                            mhT[hl][:, c, :],
                            mh_scr[hl, c].rearrange("b t d -> b (t d)"),
                        )
                        nc.sync.dma_start_transpose(
                            wT[hl][:, c, :],
                            w_scr[hl, c].rearrange("h p t d -> (h p) (t d)"),
                        )

                def hat_sweep(consume):
                    """For each group g, recompute exact fp32 hat[b, 8s, kd]
                    in PSUM (3-pass split-bf16 matmuls) and hand it to
                    `consume(g, pg)`."""
                    for g in range(NSG):
                        # one full 2KB bank per position: matmul outputs must
                        # be bank-aligned on HW
                        pgf = PSG.tile([BL, SG, 512], F32, tag="pg")
                        pg = pgf[:, :, 0:KD]
                        for u in range(SG):
                            s = g * SG + u
                            c, t = s // 2, s % 2
                            sl = slice(64 * t, 64 * t + 64)
                            o = pgf[:, u, 0:KD]
                            nc.tensor.matmul(
                                o, lhsT=mhT[0][sl, c, :], rhs=wT[0][sl, c, :],
                                start=True, stop=False, skip_group_check=True)
                            nc.tensor.matmul(
                                o, lhsT=mhT[0][sl, c, :], rhs=wT[1][sl, c, :],
                                start=False, stop=False, skip_group_check=True)
                            nc.tensor.matmul(
                                o, lhsT=mhT[1][sl, c, :], rhs=wT[0][sl, c, :],
                                start=False, stop=True, skip_group_check=True)
                        consume(g, pg)

                def delta_pass(capq, dout):
                    """dout[b,k,s] = sum_d hat[b,s,k,d]*capq[b,k,d], fp32."""
                    def consume(g, pg):
                        prod = R2.tile([BL, SG, K, D], F32, tag="prodg")
                        cq_b = capq[:].unsqueeze(1).broadcast_to([BL, SG, K, D])
                        nc.vector.tensor_tensor(
                            prod[:],
                            pg[:].rearrange("p s (k d) -> p s k d", k=K),
                            cq_b, MUL)
                        n = D
                        while n > 8:
                            h = n // 2
                            nc.vector.tensor_tensor(
                                prod[:, :, :, 0:h], prod[:, :, :, 0:h],
                                prod[:, :, :, h : 2 * h], ADD)
                            n = h
                        dv = dout[:, :, g * SG : (g + 1) * SG].rearrange(
                            "p k s -> p s k")
                        nc.vector.tensor_reduce(
                            dv, prod[:, :, :, 0:8], axis=AX.X, op=ADD)
                    hat_sweep(consume)

                def cap_pass(sw, capacc):
                    """capacc[b,k,d] = sum_s sw[b,k,s]*hat[b,s,k,d], fp32."""
                    state = {"first": True}
                    def consume(g, pg):
                        prod = R2.tile([BL, SG, K, D], F32, tag="prodg")
                        sw_src = (
                            sw[:, :, g * SG : (g + 1) * SG]
                            .rearrange("p k s -> p s k")
                            .unsqueeze(3)
                            .broadcast_to([BL, SG, K, D])
                        )
                        nc.vector.tensor_tensor(
                            prod[:],
                            pg[:].rearrange("p s (k d) -> p s k d", k=K),
                            sw_src, MUL)
                        n = SG
                        while n > 2:
                            h = n // 2
                            nc.vector.tensor_tensor(
                                prod[:, 0:h], prod[:, 0:h], prod[:, h : 2 * h], ADD)
                            n = h
                        ctmp = R.tile([BL, K, D], F32, tag="swp_ctmp")
                        pv = prod[:, 0:2].rearrange("p s k d -> p k d s")
                        nc.vector.tensor_reduce(ctmp[:], pv, axis=AX.X, op=ADD)
                        if state["first"]:
                            nc.vector.tensor_copy(capacc[:], ctmp[:])
                            state["first"] = False
                        else:
                            nc.vector.tensor_tensor(
                                capacc[:], capacc[:], ctmp[:], ADD)
                    hat_sweep(consume)

                # ---- iteration 0: cap0 = 0.25 * sum_s hat_s (exact 3-pass)
                cap0p = PSG.tile([BL, KD], F32, tag="pg")
                for ph, (la, ra) in enumerate(((0, 0), (0, 1), (1, 0))):
                    for c in range(C):
                        nc.tensor.matmul(
                            cap0p[:],
                            lhsT=mhT[la][:, c, :],
                            rhs=wT[ra][:, c, :],
                            start=(ph == 0 and c == 0),
                            stop=(ph == 2 and c == C - 1),
                            skip_group_check=True,
                        )
                nc.scalar.mul(
                    cap0s[:].rearrange("p k d -> p (k d)"), cap0p[:], 0.25
                )

                STAGE = int(os.environ.get("KBUILD_STAGE", "0"))
                if STAGE == 1:
                    nc.sync.dma_start(cap_out[:], cap0s[:])
                    nc.sync.dma_start(ro_out[:], cap0s[:, 0, :])
                    return nc
                cw = R.tile([BL, K, S], F32, tag="cw")
                f0 = _squash_factor(nc, R, cap0s, "i0")
                capq0 = _capq_from(nc, R, cap0s, f0, "i0")
                if STAGE == 15:
                    nc.sync.dma_start(cap_out[:], capq0[:])
                    nc.sync.dma_start(ro_out[:], capq0[:, 0, :])
                    return nc
                if STAGE == 16:
                    jt = R.tile([BL, KD], F32, tag="jt")
                    def consume16(g, pg):
                        if g % 2 == 0:
                            nc.vector.tensor_copy(jt[:], pg[:, 0, :])
                        else:
                            nc.scalar.copy(jt[:], pg[:, 0, :])
                    hat_sweep(consume16)
                    nc.sync.dma_start(cap_out[:], jt[:].rearrange("p (k d) -> p k d", k=K))
                    nc.sync.dma_start(ro_out[:], jt[:, 0:D])
                    return nc
                if STAGE == 17:
                    dj = R.tile([BL, K, S], F32, tag="cw")
                    delta_pass(capq0, dj)
                    nc.sync.dma_start(cap_out[:], dj[:, :, 0:D].rearrange("p k d -> p k d"))
                    nc.sync.dma_start(ro_out[:], dj[:, 0, 0:D])
                    return nc
                delta_pass(capq0, cw)

                if STAGE == 2:
                    nc.sync.dma_start(cap_out[:], capq0[:])
                    nc.sync.dma_start(ro_out[:], cw[:, 0, 0:D])
                    return nc
                sw1 = _softmax_sw(nc, R, cw, mkf, "i1")
                cap1 = R.tile([BL, K, D], F32, tag="cap1")
                cap_pass(sw1, cap1)
                f1 = _squash_factor(nc, R, cap1, "i1")
                capq1 = _capq_from(nc, R, cap1, f1, "i1")
                dtmp = R.tile([BL, K, S], F32, tag="dtmp")
                delta_pass(capq1, dtmp)
                nc.vector.tensor_tensor(cw[:], cw[:], dtmp[:], ADD)

                sw2 = _softmax_sw(nc, R, cw, mkf, "i2")
                cap2 = R.tile([BL, K, D], F32, tag="cap2")
                cap_pass(sw2, cap2)

                f2 = _squash_factor(nc, R, cap2, "i2")
                capf = R.tile([BL, K, D], F32, tag="capf")
                f2b = f2[:].unsqueeze(2).broadcast_to([BL, K, D])
                nc.vector.tensor_tensor(capf[:], cap2[:], f2b, MUL)

                # ---------------- readout: argmax_k <cap_k, eb>, gather
                att = R.tile([BL, K], F32, tag="att")
                atp = R.tile([BL, K, D], F32, tag="atp")
                eb_b = ebt[:].unsqueeze(1).broadcast_to([BL, K, D])
                nc.vector.tensor_tensor(atp[:], capf[:], eb_b, MUL)
                nc.vector.tensor_reduce(att[:], atp[:], axis=AX.X, op=ADD)
                mx4 = R.tile([BL, 1], F32, tag="mx4")
                nc.vector.tensor_reduce(mx4[:], att[:], axis=AX.X, op=MAX)
                eq = R.tile([BL, K], F32, tag="eq")
                mx4_b = mx4[:].broadcast_to([BL, K])
                nc.vector.tensor_tensor(eq[:], att[:], mx4_b, EQ)
                # first-max tie-break: sel_k = eq_k * prod_{j<k}(1-eq_j)
                ne = R.tile([BL, K], F32, tag="ne")
                nc.vector.tensor_scalar(
                    out=ne[:], in0=eq[:], scalar1=-1.0, scalar2=1.0, op0=MUL, op1=ADD
                )
                sel = R.tile([BL, K], F32, tag="sel")
                nc.vector.tensor_copy(sel[:, 0:1], eq[:, 0:1])
                pre = R.tile([BL, 1], F32, tag="pre")
                nc.vector.tensor_copy(pre[:], ne[:, 0:1])
                for k in range(1, K):
                    nc.vector.tensor_tensor(
                        sel[:, k : k + 1], eq[:, k : k + 1], pre[:], MUL)
                    if k < K - 1:
                        nc.vector.tensor_tensor(pre[:], pre[:], ne[:, k : k + 1], MUL)
                ro = R.tile([BL, D], F32, tag="ro")
                nc.vector.tensor_scalar(
                    out=ro[:], in0=capf[:, 0, :], scalar1=sel[:, 0:1], scalar2=None,
                    op0=MUL,
                )
                for k in range(1, K):
                    nc.vector.scalar_tensor_tensor(
                        out=ro[:], in0=capf[:, k, :], scalar=sel[:, k : k + 1],
                        in1=ro[:], op0=MUL, op1=ADD,
                    )

                nc.sync.dma_start(cap_out[:], capf[:])
                nc.sync.dma_start(ro_out[:], ro[:])
    return nc


_NC_CACHE = None


def _get_nc():
    global _NC_CACHE
    if _NC_CACHE is None:
        nc = build_bass()
        # split >1-sync-wait instructions for the walrus HW compile (the
        # simulator path does not tolerate the raw inserted drains)
        _TC._split_excess_waits_static(nc, max_waits=1)
        _NC_CACHE = nc
    return _NC_CACHE


def kernel(item_his_emb, item_eb, mask, w):
    from concourse.bass_utils import run_bass_kernel_spmd

    item_his_emb = np.ascontiguousarray(item_his_emb, dtype=np.float32)
    item_eb = np.ascontiguousarray(item_eb, dtype=np.float32)
    mask = np.ascontiguousarray(mask, dtype=np.int32)
    w0 = np.ascontiguousarray(np.asarray(w, dtype=np.float32)[0])

    nc = _get_nc()
    core_ids = list(range(NCORES))
    in_maps = []
    for i in core_ids:
        sl = slice(i * BL, (i + 1) * BL)
        in_maps.append(
            {
                "his": item_his_emb[sl],
                "msk": mask[sl],
                "eb": item_eb[sl],
                "w": w0,
            }
        )
    res = run_bass_kernel_spmd(nc, in_maps, core_ids)
    cap = np.concatenate([res.results[i]["cap_out"] for i in core_ids], axis=0)
    ro = np.concatenate([res.results[i]["ro_out"] for i in core_ids], axis=0)
    return cap.astype(np.float32), ro.astype(np.float32)
